# revision 13
# baseline (speedup 1.0000x reference)
"""Trainium2 Bass kernel for a ViT-style transformer block (B=32,N=577,C=768,H=12,HID=3072).

Strategy:
- Data-parallel over batch: 32 batches -> 8 cores x 4 batches.
- The execution path here (axon-tunneled PJRT) re-streams every input
  argument on every call at ~11.6 GB/s, so wall-clock per iteration is
  dominated by argument bytes, not device compute (~0.7ms on-device per the
  cost model). Two levers drive the speedup vs the f32 baseline:
    * Everything shippable is float16 (x, weights, output): rel-rounding
      ~1e-3 against a 2e-2 budget.
    * Weights are shipped sharded 1/8-per-core and AllGathered on-device
      (DRAM->DRAM collective over NeuronLink), removing the 8x data-parallel
      weight replication from the wire.
- Channel-major layout on-chip end-to-end: host pre-transposes x per batch to
  [C, N] and the weights to [K, M]; the output comes back channel-major and is
  transposed on host. This removes every on-chip transpose:
    * LayerNorm over C becomes a ones-vector matmul partition-reduction, with
      the per-token mean/rstd broadcast back across partitions via a K=1 matmul.
    * Attention computes S^T = K^T_slice . Q (keys on partitions), softmax'd
      column-wise: exp on ACT (no max subtraction needed -- |S*scale| < ~3),
      denominators via an appended ones-column on V, normalization folded into
      the PSUM->SBUF eviction against a K=1-broadcast reciprocal row.
    * The post-softmax task mask (3x3 identity block) is applied as a tiny
      rank-3 correction matmul inside the same PSUM accumulation group.
- All matmuls run fp16 x fp16 -> f32 PSUM (full PE rate, 1 cycle/row).
"""

import numpy as np

import concourse.bacc as bacc
import concourse.tile as tile
from concourse import mybir
from concourse.bass_utils import run_bass_kernel_spmd

F32 = mybir.dt.float32
F16 = mybir.dt.float16

B = 32
N = 577
C = 768
H = 12
D = 64
HID = 3072
EPS = 1e-5
SCALE = D ** -0.5

N_CORES = 8
B_PER_CORE = B // N_CORES
CT = C // 128          # 6 channel k-tiles
HT = HID // 128        # 24 hidden k-tiles
NP = 578               # token free-dim padded to even
CHUNKS = [(0, 290), (290, 288)]                    # even free-dim split of NP
MTS = [(0, 128), (128, 128), (256, 128), (384, 128), (512, 65)]  # key m-tiles (real 577)

# flat fp16 weight buffer layout (AllGathered on-device from 1/8 shards)
WQKV = C * 3 * C
WPROJ = C * C
WFC1 = C * HID
WFC2 = HID * C
WTOT = WQKV + WPROJ + WFC1 + WFC2
WSH = WTOT // N_CORES


def _layernorm_cm(nc, ps, tmp, small, src, dst, g_sb, b_sb, mm_bufs=3):
    """Channel-major layernorm: src/dst fp16 [128, CT, N]."""
    musb32 = small.tile([1, NP], F32, tag="musb32", bufs=1)
    musb = small.tile([1, NP], F16, tag="musb", bufs=1)
    varsb = small.tile([1, NP], F32, tag="varsb", bufs=1)
    rstd = small.tile([1, NP], F16, tag="rstdsb", bufs=1)
    ones_k = nc._ones_k

    for cs, cw in CHUNKS:
        sum_ps = ps.tile([1, 290], F32, tag="mm", bufs=mm_bufs)
        sq_ps = ps.tile([1, 290], F32, tag="mm", bufs=mm_bufs)
        for kt in range(CT):
            nc.tensor.matmul(sum_ps[:, :cw], ones_k, src[:, kt, cs:cs + cw],
                             start=(kt == 0), stop=(kt == CT - 1))
            xsq = tmp.tile([128, 290], F16, tag="xsq", bufs=2)
            nc.vector.tensor_mul(xsq[:, :cw], src[:, kt, cs:cs + cw], src[:, kt, cs:cs + cw])
            nc.tensor.matmul(sq_ps[:, :cw], ones_k, xsq[:, :cw],
                             start=(kt == 0), stop=(kt == CT - 1))
        nc.vector.tensor_scalar_mul(musb32[:, cs:cs + cw], sum_ps[:, :cw], 1.0 / C)
        nc.vector.tensor_copy(out=musb[:, cs:cs + cw], in_=musb32[:, cs:cs + cw])
        nc.vector.tensor_mul(varsb[:, cs:cs + cw], musb32[:, cs:cs + cw], musb32[:, cs:cs + cw])
        # var = sq/C - mu^2   (in-place: varsb holds mu^2)
        nc.vector.scalar_tensor_tensor(
            out=varsb[:, cs:cs + cw], in0=sq_ps[:, :cw], scalar=1.0 / C,
            in1=varsb[:, cs:cs + cw], op0=mybir.AluOpType.mult, op1=mybir.AluOpType.subtract)
    for cs, cw in CHUNKS:
        # rstd = 1/sqrt(var + eps), per chunk so chunk 0 unblocks early
        nc.scalar.activation(out=varsb[:, cs:cs + cw], in_=varsb[:, cs:cs + cw],
                             func=mybir.ActivationFunctionType.Sqrt,
                             bias=nc._epst[0:1, :], scale=1.0)
        nc.vector.reciprocal(out=rstd[:, cs:cs + cw], in_=varsb[:, cs:cs + cw])
        mu_ps = ps.tile([128, 290], F32, tag="mm", bufs=mm_bufs)
        rs_ps = ps.tile([128, 290], F32, tag="mm", bufs=mm_bufs)
        nc.tensor.matmul(mu_ps[:, :cw], nc._ones_b, musb[:, cs:cs + cw],
                         start=True, stop=True)
        nc.tensor.matmul(rs_ps[:, :cw], nc._ones_b, rstd[:, cs:cs + cw],
                         start=True, stop=True)
        for kt in range(CT):
            a = tmp.tile([128, 290], F32, tag="lna", bufs=2)
            nc.vector.tensor_sub(a[:, :cw], src[:, kt, cs:cs + cw], mu_ps[:, :cw])
            # (a * g) * rstd
            nc.vector.scalar_tensor_tensor(
                out=dst[:, kt, cs:cs + cw], in0=a[:, :cw], scalar=g_sb[:, kt:kt + 1],
                in1=rs_ps[:, :cw], op0=mybir.AluOpType.mult, op1=mybir.AluOpType.mult)
            nc.vector.tensor_scalar_add(dst[:, kt, cs:cs + cw], dst[:, kt, cs:cs + cw],
                                        b_sb[:, kt:kt + 1])


CV = 128 * (6 * CT + HT)
NEG = 12


def _blob_offsets(b_per_core):
    xn = b_per_core * C * N
    ocv = xn + WSH
    ong = ocv + CV
    ooc = ong + NEG
    oor = ooc + 128
    blobn = oor + 128
    return xn, ocv, ong, ooc, oor, blobn


def build_nc(b_per_core=B_PER_CORE, num_devices=N_CORES):
    nc = bacc.Bacc("TRN2", target_bir_lowering=False, debug=False,
                   num_devices=num_devices)

    # every input rides in ONE flat fp16 arg: the axon-tunneled execute path
    # costs ~1ms per argument per call, independent of size
    xn, ocv, ong, ooc, oor, blobn = _blob_offsets(b_per_core)
    blob = nc.dram_tensor("blob", [blobn], F16, kind="ExternalInput").ap()
    CN = C * N

    def xslice(b):  # [128, CT, N] channel-major view of batch b
        return blob[b * CN:(b + 1) * CN].rearrange("(kt p n) -> p kt n", p=128, n=N)

    wshard = blob[xn:xn + WSH]
    cvec_d = blob[ocv:ocv + CV].rearrange("(p k) -> p k", p=128)
    negoff_d = blob[ong:ong + NEG].rearrange("(p f) -> p f", p=3)
    onesc_d = blob[ooc:ooc + 128].rearrange("(p f) -> p f", p=128)
    onesr_d = blob[oor:oor + 128].rearrange("(p f) -> p f", p=1)
    outT = nc.dram_tensor("outT", [b_per_core, C, N], F16, kind="ExternalOutput").ap()
    x2T = nc.dram_tensor("x2T_scratch", [b_per_core, C, NP], F16).ap()
    wfull = nc.dram_tensor("wfull_gather", [WTOT], F16, addr_space="Shared").ap()

    with tile.TileContext(nc) as tc, \
         nc.allow_low_precision(reason="fp16 operands are rounded intentionally"):
        with tc.tile_pool(name="dramp", bufs=1, space="DRAM") as dramp:
            # weights arrive as a 1/8 flat shard; AllGather assembles the full
            # fp16 weight buffer in local DRAM (collectives can't touch I/O
            # tensors directly, hence the bounce copy)
            wbounce = dramp.tile([WSH], F16)
            nc.gpsimd.dma_start(out=wbounce[:], in_=wshard)
            nc.gpsimd.collective_compute(
                "AllGather", mybir.AluOpType.bypass,
                replica_groups=[list(range(num_devices))],
                ins=[wbounce.opt()], outs=[wfull])
            wqkv_v = wfull[0:WQKV].rearrange("(kt p f) -> p kt f", p=128, f=3 * C)
            wproj_v = wfull[WQKV:WQKV + WPROJ].rearrange("(kt p f) -> p kt f", p=128, f=C)
            wfc1_v = wfull[WQKV + WPROJ:WQKV + WPROJ + WFC1].rearrange(
                "(kt p f) -> p kt f", p=128, f=HID)
            wfc2_v = wfull[WQKV + WPROJ + WFC1:WTOT].rearrange(
                "(kt p f) -> p kt f", p=128, f=C)
            self_build(nc, tc, b_per_core, xslice, outT, x2T,
                       wqkv_v, wproj_v, wfc1_v, wfc2_v,
                       negoff_d, cvec_d, onesc_d, onesr_d)

    nc.compile()
    return nc


def self_build(nc, tc, b_per_core, xslice, outT, x2T,
               wqkv_v, wproj_v, wfc1_v, wfc2_v,
               negoff_d, cvec_d, onesc_d, onesr_d):
    with tc.tile_pool(name="const", bufs=1) as cst:
        ones_k = cst.tile([128, 1], F16)
        nc.sync.dma_start(out=ones_k, in_=onesc_d)
        ones_b = cst.tile([1, 128], F16)
        nc.sync.dma_start(out=ones_b, in_=onesr_d)
        ones60 = cst.tile([128, 5, H, 1], F32)
        nc.vector.memset(ones60, 1.0)
        negoff = cst.tile([3, 4], F16)   # [eye(3) - 1 | 0], loaded from host
        nc.sync.dma_start(out=negoff, in_=negoff_d)
        zeros_p = cst.tile([128, 1], F32)
        nc.vector.memset(zeros_p, 0.0)
        epst = cst.tile([1, 1], F32)
        nc.vector.memset(epst, EPS)
        nc._ones_k = ones_k
        nc._ones_b = ones_b
        nc._zeros_p = zeros_p
        nc._epst = epst

        cvech = cst.tile([128, 6 * CT + HT], F16)
        nc.sync.dma_start(out=cvech, in_=cvec_d)
        cvec = cst.tile([128, 6 * CT + HT], F32)
        nc.vector.tensor_copy(out=cvec, in_=cvech)
        g1s = cvec[:, 0 * CT:1 * CT]
        b1s = cvec[:, 1 * CT:2 * CT]
        g2s = cvec[:, 2 * CT:3 * CT]
        b2s = cvec[:, 3 * CT:4 * CT]
        bpjs = cvec[:, 4 * CT:5 * CT]
        bf2s = cvec[:, 5 * CT:6 * CT]
        bf1s = cvec[:, 6 * CT:6 * CT + HT]

        # ---------------- Phase 1: attention block ----------------
        fcw = {}
        with tc.tile_pool(name="w1", bufs=1) as w1p:
            with tc.tile_pool(name="ps1", bufs=1, space="PSUM") as ps, \
             tc.tile_pool(name="act1", bufs=1) as act, \
             tc.tile_pool(name="tmp1", bufs=1) as tmp, \
             tc.tile_pool(name="small1", bufs=1) as small:
                # x(b0) first so LN1 starts while weights gather/stream in
                xt0 = act.tile([128, CT, NP], F16, tag="xt", bufs=2, name="xt0")
                nc.vector.memset(xt0[:, :, N:NP], 0.0)
                nc.sync.dma_start(out=xt0[:, :, 0:N], in_=xslice(0))
                # all large weights share one 5-slot rotation; the fc halves
                # reuse the qkv slots once those go dead at the last batch
                wq_sb = w1p.tile([128, CT, C], F16, tag="w", bufs=5, name="wq")
                nc.sync.dma_start(out=wq_sb, in_=wqkv_v[:, :, 0:C])
                wk_sb = w1p.tile([128, CT, C], F16, tag="w", bufs=5, name="wk")
                nc.sync.dma_start(out=wk_sb, in_=wqkv_v[:, :, C:2 * C])
                wv_sb = w1p.tile([128, CT, C], F16, tag="w", bufs=5, name="wv")
                nc.sync.dma_start(out=wv_sb, in_=wqkv_v[:, :, 2 * C:3 * C])
                wproj_sb = w1p.tile([128, CT, C], F16, tag="w", bufs=5, name="wproj")
                nc.sync.dma_start(out=wproj_sb, in_=wproj_v)

                def emit_xload(b):
                    xt_ = act.tile([128, CT, NP], F16, tag="xt", bufs=2,
                                   name=f"xt_b{b}")
                    nc.vector.memset(xt_[:, :, N:NP], 0.0)
                    nc.sync.dma_start(out=xt_[:, :, 0:N], in_=xslice(b))
                    return xt_

                def emit_ln1(b, xt_):
                    ht_ = act.tile([128, CT, NP], F16, tag="ht", bufs=1,
                                   name=f"ht_b{b}")
                    _layernorm_cm(nc, ps, tmp, small, xt_, ht_, g1s, b1s)
                    return ht_

                from collections import deque

                def emit_qk_mt(ht_, qk_, mt):
                    for cs, cw in CHUNKS:
                        mm = ps.tile([128, 290], F32, tag="mm", bufs=3)
                        wqk = wq_sb if mt < CT else wk_sb
                        fo = (mt % CT) * 128
                        for kt in range(CT):
                            nc.tensor.matmul(
                                mm[:, :cw],
                                wqk[:, kt, fo:fo + 128],
                                ht_[:, kt, cs:cs + cw],
                                start=(kt == 0), stop=(kt == CT - 1))
                        if mt < CT:  # q: fold in softmax scale
                            if mt % 2 == 0:
                                nc.scalar.mul(out=qk_[:, mt, cs:cs + cw],
                                              in_=mm[:, :cw], mul=SCALE)
                            else:
                                nc.vector.tensor_scalar_mul(
                                    qk_[:, mt, cs:cs + cw], mm[:, :cw], SCALE)
                        else:
                            if mt % 2 == 0:
                                nc.scalar.copy(out=qk_[:, mt, cs:cs + cw],
                                               in_=mm[:, :cw])
                            else:
                                nc.vector.tensor_copy(
                                    out=qk_[:, mt, cs:cs + cw], in_=mm[:, :cw])

                def emit_v_mt(ht_, vaug_, imt):
                    ms, mw = MTS[imt]
                    for j in range(2):
                        vm = ps.tile([128, 384], F32, tag="mm", bufs=3)
                        for kt in range(CT):
                            nc.tensor.matmul(
                                vm[:mw, :],
                                ht_[:, kt, ms:ms + mw],
                                wv_sb[:, kt, 384 * j:384 * (j + 1)],
                                start=(kt == 0), stop=(kt == CT - 1))
                        if (imt + j) % 2 == 0:
                            nc.scalar.copy(
                                out=vaug_[:mw, imt, 6 * j:6 * (j + 1), 0:D],
                                in_=vm[:mw, :].rearrange("p (h d) -> p h d", d=D))
                        else:
                            nc.vector.tensor_copy(
                                out=vaug_[:mw, imt, 6 * j:6 * (j + 1), 0:D],
                                in_=vm[:mw, :].rearrange("p (h d) -> p h d", d=D))

                def alloc_qk(b_):
                    return act.tile([128, 2 * CT, NP], F16,
                                    tag="qk", bufs=2, name=f"qk_b{b_}")

                def alloc_vaug(b_):
                    v_ = act.tile([128, 5, H, D + 1], F16,
                                  tag="vaug", bufs=2, name=f"vaug_b{b_}")
                    nc.vector.tensor_copy(out=v_[:, :, :, D:D + 1], in_=ones60)
                    return v_

                # state carried across batches: (xt, ht, qk, vaug)
                xts = {0: xt0}
                pre = {0: emit_ln1(0, xt0)}
                qks, vaugs = {}, {}
                qks[0] = alloc_qk(0)
                for mt in range(2 * CT):
                    emit_qk_mt(pre[0], qks[0], mt)
                vaugs[0] = alloc_vaug(0)
                for imt in range(len(MTS)):
                    emit_v_mt(pre[0], vaugs[0], imt)

                for b in range(b_per_core):
                    ht = pre.pop(b)
                    xt = xts[b]
                    qk = qks.pop(b)
                    vaug = vaugs.pop(b)
                    if b + 1 < b_per_core:
                        xts[b + 1] = emit_xload(b + 1)

                    # hooks: next batch's LN/qk/v emission interleaved
                    # between this batch's attention heads
                    hooks = {}
                    if b + 1 < b_per_core:
                        def mk(fn, *args):
                            return lambda: fn(*args)
                        def hook_ln():
                            pre[b + 1] = emit_ln1(b + 1, xts[b + 1])
                        def hook_qk_alloc():
                            qks[b + 1] = alloc_qk(b + 1)
                        def hook_vaug_alloc():
                            vaugs[b + 1] = alloc_vaug(b + 1)
                        hooks[0] = [hook_ln, hook_qk_alloc]
                        for h_ in range(1, 7):
                            hooks[h_] = [mk(lambda mt_: emit_qk_mt(pre[b + 1], qks[b + 1], mt_), m)
                                         for m in (2 * (h_ - 1), 2 * (h_ - 1) + 1)]
                        hooks[7] = [hook_vaug_alloc,
                                    mk(lambda i_: emit_v_mt(pre[b + 1], vaugs[b + 1], i_), 0)]
                        hooks[8] = [mk(lambda i_: emit_v_mt(pre[b + 1], vaugs[b + 1], i_), i) for i in (1, 2)]
                        hooks[9] = [mk(lambda i_: emit_v_mt(pre[b + 1], vaugs[b + 1], i_), i) for i in (3, 4)]

                    if b == b_per_core - 1:
                        # qkv weights dead (next batch's qk/v already emitted);
                        # stream the fc weights into their slots under this
                        # attention + proj
                        NQ = 2
                        HH = HT // NQ
                        fcw["wfc1"] = [w1p.tile([128, CT, HID // NQ], F16,
                                                tag="w", bufs=5, name=f"wfc1_{i}")
                                       for i in range(NQ)]
                        fcw["wfc2"] = [w1p.tile([128, HH, C], F16,
                                                tag="w", bufs=5, name=f"wfc2_{i}")
                                       for i in range(NQ)]
                        fcw["HH"] = HH
                        for i in range(NQ):
                            lo, hi = i * (HID // NQ), (i + 1) * (HID // NQ)
                            nc.sync.dma_start(out=fcw["wfc1"][i], in_=wfc1_v[:, :, lo:hi])
                            nc.sync.dma_start(out=fcw["wfc2"][i],
                                              in_=wfc2_v[:, i * HH:(i + 1) * HH, :])

                    # attention, head by head; output channel-major into oT
                    oT = act.tile([128, CT, NP], F16, tag="oT", bufs=1)
                    pend = deque()

                    def push(fn, lag=3):
                        pend.append(fn)
                        while len(pend) > lag:
                            pend.popleft()()

                    state = {}

                    def make_o(h, imt, pt):
                        ms, mw = MTS[imt]
                        def f():
                            if "o_ps" not in state[h]:
                                state[h]["o_ps"] = [
                                    ps.tile([D + 1, 290], F32, tag="oacc", bufs=2,
                                            name=f"ops_b{b}h{h}c{ci_}")
                                    for ci_ in range(2)]
                            o_ps = state[h]["o_ps"]
                            for ci, (cs, cw) in enumerate(CHUNKS):
                                last = (imt == len(MTS) - 1)
                                nc.tensor.matmul(
                                    o_ps[ci][:, :cw],
                                    vaug[:mw, imt, h, :],
                                    pt[:mw, cs:cs + cw],
                                    start=(imt == 0), stop=last)
                            if imt == 0:
                                # task-mask correction rides inside the same
                                # accumulation group (order is commutative)
                                nc.tensor.matmul(
                                    o_ps[0][0:D, 0:4], vaug[0:3, 0, h, 0:D],
                                    state[h]["tmp33"], start=False, stop=False)
                        return f

                    def make_fin(h):
                        grp, qb = h // 2, 64 * (h % 2)
                        def f():
                            o_ps = state[h]["o_ps"]
                            for ci, (cs, cw) in enumerate(CHUNKS):
                                rsb = small.tile([1, 290], F16, tag="rsb", bufs=2)
                                nc.vector.reciprocal(out=rsb[:, :cw],
                                                     in_=o_ps[ci][D:D + 1, :cw])
                                rp = ps.tile([64, 290], F32, tag="st", bufs=3)
                                nc.tensor.matmul(rp[:, :cw], ones_b[0:1, 0:D],
                                                 rsb[:, :cw], start=True, stop=True)
                                rps = tmp.tile([64, 290], F32, tag="rps", bufs=3)
                                nc.scalar.copy(out=rps[:, :cw], in_=rp[:, :cw])
                                nc.vector.tensor_mul(oT[qb:qb + D, grp, cs:cs + cw],
                                                     o_ps[ci][0:D, :cw], rps[:, :cw])
                        return f

                    for h in range(H):
                        grp, qb = h // 2, 64 * (h % 2)
                        state[h] = {}
                        for imt, (ms, mw) in enumerate(MTS):
                            pt = tmp.tile([128, NP], F16, tag="pt", bufs=4)
                            for ci, (cs, cw) in enumerate(CHUNKS):
                                st = ps.tile([128, 290], F32, tag="st", bufs=3)
                                nc.tensor.matmul(
                                    st[:mw, :cw],
                                    qk[qb:qb + D, CT + grp, ms:ms + mw],
                                    qk[qb:qb + D, grp, cs:cs + cw],
                                    start=True, stop=True)
                                nc.scalar.activation(
                                    out=pt[:mw, cs:cs + cw], in_=st[:mw, :cw],
                                    func=mybir.ActivationFunctionType.Exp,
                                    bias=zeros_p[:mw, :], scale=1.0)
                            if imt == 0:
                                tmp33 = small.tile([3, 4], F16, tag="t33", bufs=2)
                                nc.vector.tensor_mul(tmp33, pt[0:3, 0:4], negoff)
                                state[h]["tmp33"] = tmp33
                            push(make_o(h, imt, pt))
                        push(make_fin(h))
                        for fn in hooks.get(h, []):
                            fn()
                    while pend:
                        pend.popleft()()

                    # proj + bias + residual -> x2T (DRAM scratch)
                    for mt in range(CT):
                        for cs, cw in CHUNKS:
                            mm = ps.tile([128, 290], F32, tag="mm", bufs=3)
                            for kt in range(CT):
                                nc.tensor.matmul(
                                    mm[:, :cw],
                                    wproj_sb[:, kt, mt * 128:(mt + 1) * 128],
                                    oT[:, kt, cs:cs + cw],
                                    start=(kt == 0), stop=(kt == CT - 1))
                            x2c = tmp.tile([128, 290], F16, tag="x2c", bufs=2)
                            nc.vector.scalar_tensor_tensor(
                                out=x2c[:, :cw], in0=mm[:, :cw],
                                scalar=bpjs[:, mt:mt + 1], in1=xt[:, mt, cs:cs + cw],
                                op0=mybir.AluOpType.add, op1=mybir.AluOpType.add)
                            nc.sync.dma_start(
                                out=x2T[b].rearrange("(kt p) n -> p kt n", p=128)[:, mt, cs:cs + cw],
                                in_=x2c[:, :cw])

            # ---------------- Phase 2: MLP block ----------------
            with tc.tile_pool(name="ps2", bufs=1, space="PSUM") as ps, \
             tc.tile_pool(name="act2", bufs=1) as act, \
             tc.tile_pool(name="tmp2", bufs=1) as tmp, \
             tc.tile_pool(name="small2", bufs=1) as small:
                wfc1_sb, wfc2_sb, HH = fcw["wfc1"], fcw["wfc2"], fcw["HH"]
                x2t0 = act.tile([128, CT, NP], F16, tag="x2t", bufs=2, name="x2t0")
                nc.sync.dma_start(out=x2t0, in_=x2T[0].rearrange("(kt p) n -> p kt n", p=128))
                h2t0 = act.tile([128, CT, NP], F16, tag="h2t", bufs=2, name="h2t0")
                _layernorm_cm(nc, ps, tmp, small, x2t0, h2t0, g2s, b2s, mm_bufs=2)

                def emit_x2load(b):
                    x2t_ = act.tile([128, CT, NP], F16, tag="x2t", bufs=2,
                                    name=f"x2t_b{b}")
                    nc.sync.dma_start(out=x2t_, in_=x2T[b].rearrange("(kt p) n -> p kt n", p=128))
                    return x2t_

                def emit_ln2(b, x2t_):
                    h2t_ = act.tile([128, CT, NP], F16, tag="h2t",
                                    bufs=2, name=f"h2t_b{b}")
                    _layernorm_cm(nc, ps, tmp, small, x2t_, h2t_, g2s, b2s, mm_bufs=2)
                    return h2t_

                x2ts = {0: x2t0}
                pre2 = {0: (x2t0, h2t0)}
                for b in range(b_per_core):
                    x2t, h2t = pre2.pop(b)
                    if b + 1 < b_per_core:
                        x2ts[b + 1] = emit_x2load(b + 1)

                    for ci_chunk, (cs, cw) in enumerate(CHUNKS):
                        # LN2 of the next batch rides under this batch's last chunk
                        if ci_chunk == 1 and b + 1 < b_per_core:
                            pre2[b + 1] = (x2ts[b + 1], emit_ln2(b + 1, x2ts[b + 1]))
                        f2ps = [ps.tile([128, 290], F32, tag="fc2", bufs=6,
                                        name=f"f2ps_b{b}c{cs}m{mt_}")
                                for mt_ in range(CT)]
                        for kt in range(HT):
                            f1 = ps.tile([128, 290], F32, tag="mm", bufs=2)
                            w1piece = wfc1_sb[kt // HH]
                            ko = (kt % HH) * 128
                            for ct in range(CT):
                                nc.tensor.matmul(
                                    f1[:, :cw],
                                    w1piece[:, ct, ko:ko + 128],
                                    h2t[:, ct, cs:cs + cw],
                                    start=(ct == 0), stop=(ct == CT - 1))
                            h3 = tmp.tile([128, 290], F16, tag="h3", bufs=3)
                            nc.scalar.activation(
                                out=h3[:, :cw], in_=f1[:, :cw],
                                func=mybir.ActivationFunctionType.Gelu,
                                bias=bf1s[:, kt:kt + 1], scale=1.0)
                            w2piece = wfc2_sb[kt // HH]
                            for mt in range(CT):
                                nc.tensor.matmul(
                                    f2ps[mt][:, :cw],
                                    w2piece[:, kt % HH, mt * 128:(mt + 1) * 128],
                                    h3[:, :cw],
                                    start=(kt == 0), stop=(kt == HT - 1))
                        for mt in range(CT):
                            outc = tmp.tile([128, 290], F16, tag="outc", bufs=3)
                            nc.vector.scalar_tensor_tensor(
                                out=outc[:, :cw], in0=f2ps[mt][:, :cw],
                                scalar=bf2s[:, mt:mt + 1], in1=x2t[:, mt, cs:cs + cw],
                                op0=mybir.AluOpType.add, op1=mybir.AluOpType.add)
                            wout = min(cs + cw, N) - cs
                            nc.sync.dma_start(
                                out=outT[b].rearrange("(kt p) n -> p kt n", p=128)[:, mt, cs:cs + wout],
                                in_=outc[:, :wout])


_NC_CACHE = {}


def _get_nc(b_per_core=B_PER_CORE, num_devices=N_CORES):
    key = (b_per_core, num_devices)
    if key not in _NC_CACHE:
        _NC_CACHE[key] = build_nc(b_per_core, num_devices)
    return _NC_CACHE[key]


def make_in_maps(x, w_qkv, w_proj, b_proj, ln1_g, ln1_b, ln2_g, ln2_b,
                 w_fc1, b_fc1, w_fc2, b_fc2, b_per_core=B_PER_CORE,
                 num_devices=N_CORES):
    f16 = np.float16
    xT = np.ascontiguousarray(
        np.asarray(x, dtype=np.float32).transpose(0, 2, 1)).astype(f16)
    wflat = np.concatenate([
        np.ascontiguousarray(np.asarray(w_qkv, np.float32).T).astype(f16).reshape(-1),
        np.ascontiguousarray(np.asarray(w_proj, np.float32).T).astype(f16).reshape(-1),
        np.ascontiguousarray(np.asarray(w_fc1, np.float32).T).astype(f16).reshape(-1),
        np.ascontiguousarray(np.asarray(w_fc2, np.float32).T).astype(f16).reshape(-1),
    ])
    assert wflat.shape[0] == WTOT
    wshards = wflat.reshape(num_devices, -1)
    cvec = np.concatenate(
        [np.asarray(v, np.float32).reshape(-1, 128).T
         for v in (ln1_g, ln1_b, ln2_g, ln2_b, b_proj, b_fc2, b_fc1)],
        axis=1).astype(f16)
    negoff = np.concatenate([np.eye(3) - 1.0, np.zeros((3, 1))], 1).astype(f16)
    tail = np.concatenate([cvec.reshape(-1), negoff.reshape(-1),
                           np.ones(256, f16)])
    return [
        {"blob": np.concatenate([
            xT[i * b_per_core:(i + 1) * b_per_core].reshape(-1),
            wshards[i], tail])}
        for i in range(num_devices)
    ]


def kernel(x, w_qkv, w_proj, b_proj, ln1_g, ln1_b, ln2_g, ln2_b,
           w_fc1, b_fc1, w_fc2, b_fc2):
    nc = _get_nc()
    in_maps = make_in_maps(x, w_qkv, w_proj, b_proj, ln1_g, ln1_b, ln2_g, ln2_b,
                           w_fc1, b_fc1, w_fc2, b_fc2)
    res = run_bass_kernel_spmd(nc, in_maps, core_ids=list(range(N_CORES)))
    outT = np.concatenate([r["outT"] for r in res.results], axis=0)  # [B, C, N] f16
    return np.ascontiguousarray(outT.transpose(0, 2, 1)).astype(np.float32)


# revision 15
# speedup vs baseline: 2.3159x; 2.3159x over previous
"""Trainium2 Bass kernel for a ViT-style transformer block (B=32,N=577,C=768,H=12,HID=3072).

Strategy:
- Data-parallel over batch: 32 batches -> 8 cores x 4 batches.
- The execution path here (axon-tunneled PJRT) re-streams every input
  argument on every call at ~11.6 GB/s, so wall-clock per iteration is
  dominated by argument bytes, not device compute (~0.7ms on-device per the
  cost model). Two levers drive the speedup vs the f32 baseline:
    * Everything shippable is float16 (x, weights, output): rel-rounding
      ~1e-3 against a 2e-2 budget.
    * Weights are shipped sharded 1/8-per-core and AllGathered on-device
      (DRAM->DRAM collective over NeuronLink), removing the 8x data-parallel
      weight replication from the wire.
- Channel-major layout on-chip end-to-end: host pre-transposes x per batch to
  [C, N] and the weights to [K, M]; the output comes back channel-major and is
  transposed on host. This removes every on-chip transpose:
    * LayerNorm over C becomes a ones-vector matmul partition-reduction, with
      the per-token mean/rstd broadcast back across partitions via a K=1 matmul.
    * Attention computes S^T = K^T_slice . Q (keys on partitions), softmax'd
      column-wise: exp on ACT (no max subtraction needed -- |S*scale| < ~3),
      denominators via an appended ones-column on V, normalization folded into
      the PSUM->SBUF eviction against a K=1-broadcast reciprocal row.
    * The post-softmax task mask (3x3 identity block) is applied as a tiny
      rank-3 correction matmul inside the same PSUM accumulation group.
- All matmuls run fp16 x fp16 -> f32 PSUM (full PE rate, 1 cycle/row).
"""

import numpy as np

import concourse.bacc as bacc
import concourse.tile as tile
from concourse import mybir
from concourse.bass_utils import run_bass_kernel_spmd

F32 = mybir.dt.float32
F16 = mybir.dt.float16

B = 32
N = 577
C = 768
H = 12
D = 64
HID = 3072
EPS = 1e-5
SCALE = D ** -0.5

N_CORES = 8
B_PER_CORE = B // N_CORES
CT = C // 128          # 6 channel k-tiles
HT = HID // 128        # 24 hidden k-tiles
NP = 578               # token free-dim padded to even
CHUNKS = [(0, 290), (290, 288)]                    # even free-dim split of NP
MTS = [(0, 128), (128, 128), (256, 128), (384, 128), (512, 65)]  # key m-tiles (real 577)

# flat fp16 weight buffer layout (AllGathered on-device from 1/8 shards)
WQKV = C * 3 * C
WPROJ = C * C
WFC1 = C * HID
WFC2 = HID * C
WTOT = WQKV + WPROJ + WFC1 + WFC2
WSH = WTOT // N_CORES


def _layernorm_cm(nc, ps, tmp, small, src, dst, g_sb, b_sb, mm_bufs=3):
    """Channel-major layernorm: src/dst fp16 [128, CT, N]."""
    musb32 = small.tile([1, NP], F32, tag="musb32", bufs=1)
    musb = small.tile([1, NP], F16, tag="musb", bufs=1)
    varsb = small.tile([1, NP], F32, tag="varsb", bufs=1)
    rstd = small.tile([1, NP], F16, tag="rstdsb", bufs=1)
    ones_k = nc._ones_k

    for cs, cw in CHUNKS:
        sum_ps = ps.tile([1, 290], F32, tag="mm", bufs=mm_bufs)
        sq_ps = ps.tile([1, 290], F32, tag="mm", bufs=mm_bufs)
        for kt in range(CT):
            nc.tensor.matmul(sum_ps[:, :cw], ones_k, src[:, kt, cs:cs + cw],
                             start=(kt == 0), stop=(kt == CT - 1))
            xsq = tmp.tile([128, 290], F16, tag="xsq", bufs=2)
            nc.vector.tensor_mul(xsq[:, :cw], src[:, kt, cs:cs + cw], src[:, kt, cs:cs + cw])
            nc.tensor.matmul(sq_ps[:, :cw], ones_k, xsq[:, :cw],
                             start=(kt == 0), stop=(kt == CT - 1))
        nc.vector.tensor_scalar_mul(musb32[:, cs:cs + cw], sum_ps[:, :cw], 1.0 / C)
        nc.vector.tensor_copy(out=musb[:, cs:cs + cw], in_=musb32[:, cs:cs + cw])
        nc.vector.tensor_mul(varsb[:, cs:cs + cw], musb32[:, cs:cs + cw], musb32[:, cs:cs + cw])
        # var = sq/C - mu^2   (in-place: varsb holds mu^2)
        nc.vector.scalar_tensor_tensor(
            out=varsb[:, cs:cs + cw], in0=sq_ps[:, :cw], scalar=1.0 / C,
            in1=varsb[:, cs:cs + cw], op0=mybir.AluOpType.mult, op1=mybir.AluOpType.subtract)
    for cs, cw in CHUNKS:
        # rstd = 1/sqrt(var + eps), per chunk so chunk 0 unblocks early
        nc.scalar.activation(out=varsb[:, cs:cs + cw], in_=varsb[:, cs:cs + cw],
                             func=mybir.ActivationFunctionType.Sqrt,
                             bias=nc._epst[0:1, :], scale=1.0)
        nc.vector.reciprocal(out=rstd[:, cs:cs + cw], in_=varsb[:, cs:cs + cw])
        mu_ps = ps.tile([128, 290], F32, tag="mm", bufs=mm_bufs)
        rs_ps = ps.tile([128, 290], F32, tag="mm", bufs=mm_bufs)
        nc.tensor.matmul(mu_ps[:, :cw], nc._ones_b, musb[:, cs:cs + cw],
                         start=True, stop=True)
        nc.tensor.matmul(rs_ps[:, :cw], nc._ones_b, rstd[:, cs:cs + cw],
                         start=True, stop=True)
        for kt in range(CT):
            a = tmp.tile([128, 290], F32, tag="lna", bufs=2)
            nc.vector.tensor_sub(a[:, :cw], src[:, kt, cs:cs + cw], mu_ps[:, :cw])
            # (a * g) * rstd
            nc.vector.scalar_tensor_tensor(
                out=dst[:, kt, cs:cs + cw], in0=a[:, :cw], scalar=g_sb[:, kt:kt + 1],
                in1=rs_ps[:, :cw], op0=mybir.AluOpType.mult, op1=mybir.AluOpType.mult)
            nc.vector.tensor_scalar_add(dst[:, kt, cs:cs + cw], dst[:, kt, cs:cs + cw],
                                        b_sb[:, kt:kt + 1])


CV = 128 * (6 * CT + HT)
NEG = 12
B2N = WSH + CV + NEG + 256  # weights-shard + constants blob


def build_nc(b_per_core=B_PER_CORE, num_devices=N_CORES):
    nc = bacc.Bacc("TRN2", target_bir_lowering=False, debug=False,
                   num_devices=num_devices)

    # inputs ride in TWO flat fp16 args: the axon-tunneled execute path costs
    # ~1.1ms of handling per argument per call, and any single argument over
    # ~4MiB falls onto a much slower streaming path -- so pack everything into
    # as few sub-4MiB args as possible
    xn = b_per_core * C * N
    xa = nc.dram_tensor("xa", [xn], F16, kind="ExternalInput").ap()
    blob2 = nc.dram_tensor("blob2", [B2N], F16, kind="ExternalInput").ap()
    CN = C * N

    def xslice(b):  # [128, CT, N] channel-major view of batch b
        return xa[b * CN:(b + 1) * CN].rearrange("(kt p n) -> p kt n", p=128, n=N)

    wshard = blob2[0:WSH]
    cvec_d = blob2[WSH:WSH + CV].rearrange("(p k) -> p k", p=128)
    negoff_d = blob2[WSH + CV:WSH + CV + NEG].rearrange("(p f) -> p f", p=3)
    onesc_d = blob2[WSH + CV + NEG:WSH + CV + NEG + 128].rearrange("(p f) -> p f", p=128)
    onesr_d = blob2[WSH + CV + NEG + 128:WSH + CV + NEG + 256].rearrange("(p f) -> p f", p=1)
    outT = nc.dram_tensor("outT", [b_per_core, C, N], F16, kind="ExternalOutput").ap()
    x2T = nc.dram_tensor("x2T_scratch", [b_per_core, C, NP], F16).ap()
    wfull = nc.dram_tensor("wfull_gather", [WTOT], F16, addr_space="Shared").ap()

    with tile.TileContext(nc) as tc, \
         nc.allow_low_precision(reason="fp16 operands are rounded intentionally"):
        with tc.tile_pool(name="dramp", bufs=1, space="DRAM") as dramp:
            # weights arrive as a 1/8 flat shard; AllGather assembles the full
            # fp16 weight buffer in local DRAM (collectives can't touch I/O
            # tensors directly, hence the bounce copy)
            wbounce = dramp.tile([WSH], F16)
            nc.gpsimd.dma_start(out=wbounce[:], in_=wshard)
            nc.gpsimd.collective_compute(
                "AllGather", mybir.AluOpType.bypass,
                replica_groups=[list(range(num_devices))],
                ins=[wbounce.opt()], outs=[wfull])
            wqkv_v = wfull[0:WQKV].rearrange("(kt p f) -> p kt f", p=128, f=3 * C)
            wproj_v = wfull[WQKV:WQKV + WPROJ].rearrange("(kt p f) -> p kt f", p=128, f=C)
            wfc1_v = wfull[WQKV + WPROJ:WQKV + WPROJ + WFC1].rearrange(
                "(kt p f) -> p kt f", p=128, f=HID)
            wfc2_v = wfull[WQKV + WPROJ + WFC1:WTOT].rearrange(
                "(kt p f) -> p kt f", p=128, f=C)
            self_build(nc, tc, b_per_core, xslice, outT, x2T,
                       wqkv_v, wproj_v, wfc1_v, wfc2_v,
                       negoff_d, cvec_d, onesc_d, onesr_d)

    nc.compile()
    return nc


def self_build(nc, tc, b_per_core, xslice, outT, x2T,
               wqkv_v, wproj_v, wfc1_v, wfc2_v,
               negoff_d, cvec_d, onesc_d, onesr_d):
    with tc.tile_pool(name="const", bufs=1) as cst:
        ones_k = cst.tile([128, 1], F16)
        nc.sync.dma_start(out=ones_k, in_=onesc_d)
        ones_b = cst.tile([1, 128], F16)
        nc.sync.dma_start(out=ones_b, in_=onesr_d)
        ones60 = cst.tile([128, 5, H, 1], F32)
        nc.vector.memset(ones60, 1.0)
        negoff = cst.tile([3, 4], F16)   # [eye(3) - 1 | 0], loaded from host
        nc.sync.dma_start(out=negoff, in_=negoff_d)
        zeros_p = cst.tile([128, 1], F32)
        nc.vector.memset(zeros_p, 0.0)
        epst = cst.tile([1, 1], F32)
        nc.vector.memset(epst, EPS)
        nc._ones_k = ones_k
        nc._ones_b = ones_b
        nc._zeros_p = zeros_p
        nc._epst = epst

        cvech = cst.tile([128, 6 * CT + HT], F16)
        nc.sync.dma_start(out=cvech, in_=cvec_d)
        cvec = cst.tile([128, 6 * CT + HT], F32)
        nc.vector.tensor_copy(out=cvec, in_=cvech)
        g1s = cvec[:, 0 * CT:1 * CT]
        b1s = cvec[:, 1 * CT:2 * CT]
        g2s = cvec[:, 2 * CT:3 * CT]
        b2s = cvec[:, 3 * CT:4 * CT]
        bpjs = cvec[:, 4 * CT:5 * CT]
        bf2s = cvec[:, 5 * CT:6 * CT]
        bf1s = cvec[:, 6 * CT:6 * CT + HT]

        # ---------------- Phase 1: attention block ----------------
        fcw = {}
        with tc.tile_pool(name="w1", bufs=1) as w1p:
            with tc.tile_pool(name="ps1", bufs=1, space="PSUM") as ps, \
             tc.tile_pool(name="act1", bufs=1) as act, \
             tc.tile_pool(name="tmp1", bufs=1) as tmp, \
             tc.tile_pool(name="small1", bufs=1) as small:
                # x(b0) first so LN1 starts while weights gather/stream in
                xt0 = act.tile([128, CT, NP], F16, tag="xt", bufs=2, name="xt0")
                nc.vector.memset(xt0[:, :, N:NP], 0.0)
                nc.sync.dma_start(out=xt0[:, :, 0:N], in_=xslice(0))
                # all large weights share one 5-slot rotation; the fc halves
                # reuse the qkv slots once those go dead at the last batch
                wq_sb = w1p.tile([128, CT, C], F16, tag="w", bufs=5, name="wq")
                nc.sync.dma_start(out=wq_sb, in_=wqkv_v[:, :, 0:C])
                wk_sb = w1p.tile([128, CT, C], F16, tag="w", bufs=5, name="wk")
                nc.sync.dma_start(out=wk_sb, in_=wqkv_v[:, :, C:2 * C])
                wv_sb = w1p.tile([128, CT, C], F16, tag="w", bufs=5, name="wv")
                nc.sync.dma_start(out=wv_sb, in_=wqkv_v[:, :, 2 * C:3 * C])
                wproj_sb = w1p.tile([128, CT, C], F16, tag="w", bufs=5, name="wproj")
                nc.sync.dma_start(out=wproj_sb, in_=wproj_v)

                def emit_xload(b):
                    xt_ = act.tile([128, CT, NP], F16, tag="xt", bufs=2,
                                   name=f"xt_b{b}")
                    nc.vector.memset(xt_[:, :, N:NP], 0.0)
                    nc.sync.dma_start(out=xt_[:, :, 0:N], in_=xslice(b))
                    return xt_

                def emit_ln1(b, xt_):
                    ht_ = act.tile([128, CT, NP], F16, tag="ht", bufs=1,
                                   name=f"ht_b{b}")
                    _layernorm_cm(nc, ps, tmp, small, xt_, ht_, g1s, b1s)
                    return ht_

                from collections import deque

                def emit_qk_mt(ht_, qk_, mt):
                    for cs, cw in CHUNKS:
                        mm = ps.tile([128, 290], F32, tag="mm", bufs=3)
                        wqk = wq_sb if mt < CT else wk_sb
                        fo = (mt % CT) * 128
                        for kt in range(CT):
                            nc.tensor.matmul(
                                mm[:, :cw],
                                wqk[:, kt, fo:fo + 128],
                                ht_[:, kt, cs:cs + cw],
                                start=(kt == 0), stop=(kt == CT - 1))
                        if mt < CT:  # q: fold in softmax scale
                            if mt % 2 == 0:
                                nc.scalar.mul(out=qk_[:, mt, cs:cs + cw],
                                              in_=mm[:, :cw], mul=SCALE)
                            else:
                                nc.vector.tensor_scalar_mul(
                                    qk_[:, mt, cs:cs + cw], mm[:, :cw], SCALE)
                        else:
                            if mt % 2 == 0:
                                nc.scalar.copy(out=qk_[:, mt, cs:cs + cw],
                                               in_=mm[:, :cw])
                            else:
                                nc.vector.tensor_copy(
                                    out=qk_[:, mt, cs:cs + cw], in_=mm[:, :cw])

                def emit_v_mt(ht_, vaug_, imt):
                    ms, mw = MTS[imt]
                    for j in range(2):
                        vm = ps.tile([128, 384], F32, tag="mm", bufs=3)
                        for kt in range(CT):
                            nc.tensor.matmul(
                                vm[:mw, :],
                                ht_[:, kt, ms:ms + mw],
                                wv_sb[:, kt, 384 * j:384 * (j + 1)],
                                start=(kt == 0), stop=(kt == CT - 1))
                        if (imt + j) % 2 == 0:
                            nc.scalar.copy(
                                out=vaug_[:mw, imt, 6 * j:6 * (j + 1), 0:D],
                                in_=vm[:mw, :].rearrange("p (h d) -> p h d", d=D))
                        else:
                            nc.vector.tensor_copy(
                                out=vaug_[:mw, imt, 6 * j:6 * (j + 1), 0:D],
                                in_=vm[:mw, :].rearrange("p (h d) -> p h d", d=D))

                def alloc_qk(b_):
                    return act.tile([128, 2 * CT, NP], F16,
                                    tag="qk", bufs=2, name=f"qk_b{b_}")

                def alloc_vaug(b_):
                    v_ = act.tile([128, 5, H, D + 1], F16,
                                  tag="vaug", bufs=2, name=f"vaug_b{b_}")
                    nc.vector.tensor_copy(out=v_[:, :, :, D:D + 1], in_=ones60)
                    return v_

                # state carried across batches: (xt, ht, qk, vaug)
                xts = {0: xt0}
                pre = {0: emit_ln1(0, xt0)}
                qks, vaugs = {}, {}
                qks[0] = alloc_qk(0)
                for mt in range(2 * CT):
                    emit_qk_mt(pre[0], qks[0], mt)
                vaugs[0] = alloc_vaug(0)
                for imt in range(len(MTS)):
                    emit_v_mt(pre[0], vaugs[0], imt)

                for b in range(b_per_core):
                    ht = pre.pop(b)
                    xt = xts[b]
                    qk = qks.pop(b)
                    vaug = vaugs.pop(b)
                    if b + 1 < b_per_core:
                        xts[b + 1] = emit_xload(b + 1)

                    # hooks: next batch's LN/qk/v emission interleaved
                    # between this batch's attention heads
                    hooks = {}
                    if b + 1 < b_per_core:
                        def mk(fn, *args):
                            return lambda: fn(*args)
                        def hook_ln():
                            pre[b + 1] = emit_ln1(b + 1, xts[b + 1])
                        def hook_qk_alloc():
                            qks[b + 1] = alloc_qk(b + 1)
                        def hook_vaug_alloc():
                            vaugs[b + 1] = alloc_vaug(b + 1)
                        hooks[0] = [hook_ln, hook_qk_alloc]
                        for h_ in range(1, 7):
                            hooks[h_] = [mk(lambda mt_: emit_qk_mt(pre[b + 1], qks[b + 1], mt_), m)
                                         for m in (2 * (h_ - 1), 2 * (h_ - 1) + 1)]
                        hooks[7] = [hook_vaug_alloc,
                                    mk(lambda i_: emit_v_mt(pre[b + 1], vaugs[b + 1], i_), 0)]
                        hooks[8] = [mk(lambda i_: emit_v_mt(pre[b + 1], vaugs[b + 1], i_), i) for i in (1, 2)]
                        hooks[9] = [mk(lambda i_: emit_v_mt(pre[b + 1], vaugs[b + 1], i_), i) for i in (3, 4)]

                    if b == b_per_core - 1:
                        # qkv weights dead (next batch's qk/v already emitted);
                        # stream the fc weights into their slots under this
                        # attention + proj
                        NQ = 2
                        HH = HT // NQ
                        fcw["wfc1"] = [w1p.tile([128, CT, HID // NQ], F16,
                                                tag="w", bufs=5, name=f"wfc1_{i}")
                                       for i in range(NQ)]
                        fcw["wfc2"] = [w1p.tile([128, HH, C], F16,
                                                tag="w", bufs=5, name=f"wfc2_{i}")
                                       for i in range(NQ)]
                        fcw["HH"] = HH
                        for i in range(NQ):
                            lo, hi = i * (HID // NQ), (i + 1) * (HID // NQ)
                            nc.sync.dma_start(out=fcw["wfc1"][i], in_=wfc1_v[:, :, lo:hi])
                            nc.sync.dma_start(out=fcw["wfc2"][i],
                                              in_=wfc2_v[:, i * HH:(i + 1) * HH, :])

                    # attention, head by head; output channel-major into oT
                    oT = act.tile([128, CT, NP], F16, tag="oT", bufs=1)
                    pend = deque()

                    def push(fn, lag=3):
                        pend.append(fn)
                        while len(pend) > lag:
                            pend.popleft()()

                    state = {}

                    def make_o(h, imt, pt):
                        ms, mw = MTS[imt]
                        def f():
                            if "o_ps" not in state[h]:
                                state[h]["o_ps"] = [
                                    ps.tile([D + 1, 290], F32, tag="oacc", bufs=2,
                                            name=f"ops_b{b}h{h}c{ci_}")
                                    for ci_ in range(2)]
                            o_ps = state[h]["o_ps"]
                            for ci, (cs, cw) in enumerate(CHUNKS):
                                last = (imt == len(MTS) - 1)
                                nc.tensor.matmul(
                                    o_ps[ci][:, :cw],
                                    vaug[:mw, imt, h, :],
                                    pt[:mw, cs:cs + cw],
                                    start=(imt == 0), stop=last)
                            if imt == 0:
                                # task-mask correction rides inside the same
                                # accumulation group (order is commutative)
                                nc.tensor.matmul(
                                    o_ps[0][0:D, 0:4], vaug[0:3, 0, h, 0:D],
                                    state[h]["tmp33"], start=False, stop=False)
                        return f

                    def make_fin(h):
                        grp, qb = h // 2, 64 * (h % 2)
                        def f():
                            o_ps = state[h]["o_ps"]
                            for ci, (cs, cw) in enumerate(CHUNKS):
                                rsb = small.tile([1, 290], F16, tag="rsb", bufs=2)
                                nc.vector.reciprocal(out=rsb[:, :cw],
                                                     in_=o_ps[ci][D:D + 1, :cw])
                                rp = ps.tile([64, 290], F32, tag="st", bufs=3)
                                nc.tensor.matmul(rp[:, :cw], ones_b[0:1, 0:D],
                                                 rsb[:, :cw], start=True, stop=True)
                                rps = tmp.tile([64, 290], F32, tag="rps", bufs=3)
                                nc.scalar.copy(out=rps[:, :cw], in_=rp[:, :cw])
                                nc.vector.tensor_mul(oT[qb:qb + D, grp, cs:cs + cw],
                                                     o_ps[ci][0:D, :cw], rps[:, :cw])
                        return f

                    for h in range(H):
                        grp, qb = h // 2, 64 * (h % 2)
                        state[h] = {}
                        for imt, (ms, mw) in enumerate(MTS):
                            pt = tmp.tile([128, NP], F16, tag="pt", bufs=4)
                            for ci, (cs, cw) in enumerate(CHUNKS):
                                st = ps.tile([128, 290], F32, tag="st", bufs=3)
                                nc.tensor.matmul(
                                    st[:mw, :cw],
                                    qk[qb:qb + D, CT + grp, ms:ms + mw],
                                    qk[qb:qb + D, grp, cs:cs + cw],
                                    start=True, stop=True)
                                nc.scalar.activation(
                                    out=pt[:mw, cs:cs + cw], in_=st[:mw, :cw],
                                    func=mybir.ActivationFunctionType.Exp,
                                    bias=zeros_p[:mw, :], scale=1.0)
                            if imt == 0:
                                tmp33 = small.tile([3, 4], F16, tag="t33", bufs=2)
                                nc.vector.tensor_mul(tmp33, pt[0:3, 0:4], negoff)
                                state[h]["tmp33"] = tmp33
                            push(make_o(h, imt, pt))
                        push(make_fin(h))
                        for fn in hooks.get(h, []):
                            fn()
                    while pend:
                        pend.popleft()()

                    # proj + bias + residual -> x2T (DRAM scratch)
                    for mt in range(CT):
                        for cs, cw in CHUNKS:
                            mm = ps.tile([128, 290], F32, tag="mm", bufs=3)
                            for kt in range(CT):
                                nc.tensor.matmul(
                                    mm[:, :cw],
                                    wproj_sb[:, kt, mt * 128:(mt + 1) * 128],
                                    oT[:, kt, cs:cs + cw],
                                    start=(kt == 0), stop=(kt == CT - 1))
                            x2c = tmp.tile([128, 290], F16, tag="x2c", bufs=2)
                            nc.vector.scalar_tensor_tensor(
                                out=x2c[:, :cw], in0=mm[:, :cw],
                                scalar=bpjs[:, mt:mt + 1], in1=xt[:, mt, cs:cs + cw],
                                op0=mybir.AluOpType.add, op1=mybir.AluOpType.add)
                            nc.sync.dma_start(
                                out=x2T[b].rearrange("(kt p) n -> p kt n", p=128)[:, mt, cs:cs + cw],
                                in_=x2c[:, :cw])

            # ---------------- Phase 2: MLP block ----------------
            with tc.tile_pool(name="ps2", bufs=1, space="PSUM") as ps, \
             tc.tile_pool(name="act2", bufs=1) as act, \
             tc.tile_pool(name="tmp2", bufs=1) as tmp, \
             tc.tile_pool(name="small2", bufs=1) as small:
                wfc1_sb, wfc2_sb, HH = fcw["wfc1"], fcw["wfc2"], fcw["HH"]
                x2t0 = act.tile([128, CT, NP], F16, tag="x2t", bufs=2, name="x2t0")
                nc.sync.dma_start(out=x2t0, in_=x2T[0].rearrange("(kt p) n -> p kt n", p=128))
                h2t0 = act.tile([128, CT, NP], F16, tag="h2t", bufs=2, name="h2t0")
                _layernorm_cm(nc, ps, tmp, small, x2t0, h2t0, g2s, b2s, mm_bufs=2)

                def emit_x2load(b):
                    x2t_ = act.tile([128, CT, NP], F16, tag="x2t", bufs=2,
                                    name=f"x2t_b{b}")
                    nc.sync.dma_start(out=x2t_, in_=x2T[b].rearrange("(kt p) n -> p kt n", p=128))
                    return x2t_

                def emit_ln2(b, x2t_):
                    h2t_ = act.tile([128, CT, NP], F16, tag="h2t",
                                    bufs=2, name=f"h2t_b{b}")
                    _layernorm_cm(nc, ps, tmp, small, x2t_, h2t_, g2s, b2s, mm_bufs=2)
                    return h2t_

                x2ts = {0: x2t0}
                pre2 = {0: (x2t0, h2t0)}
                for b in range(b_per_core):
                    x2t, h2t = pre2.pop(b)
                    if b + 1 < b_per_core:
                        x2ts[b + 1] = emit_x2load(b + 1)

                    for ci_chunk, (cs, cw) in enumerate(CHUNKS):
                        # LN2 of the next batch rides under this batch's last chunk
                        if ci_chunk == 1 and b + 1 < b_per_core:
                            pre2[b + 1] = (x2ts[b + 1], emit_ln2(b + 1, x2ts[b + 1]))
                        f2ps = [ps.tile([128, 290], F32, tag="fc2", bufs=6,
                                        name=f"f2ps_b{b}c{cs}m{mt_}")
                                for mt_ in range(CT)]
                        for kt in range(HT):
                            f1 = ps.tile([128, 290], F32, tag="mm", bufs=2)
                            w1piece = wfc1_sb[kt // HH]
                            ko = (kt % HH) * 128
                            for ct in range(CT):
                                nc.tensor.matmul(
                                    f1[:, :cw],
                                    w1piece[:, ct, ko:ko + 128],
                                    h2t[:, ct, cs:cs + cw],
                                    start=(ct == 0), stop=(ct == CT - 1))
                            h3 = tmp.tile([128, 290], F16, tag="h3", bufs=3)
                            nc.scalar.activation(
                                out=h3[:, :cw], in_=f1[:, :cw],
                                func=mybir.ActivationFunctionType.Gelu,
                                bias=bf1s[:, kt:kt + 1], scale=1.0)
                            w2piece = wfc2_sb[kt // HH]
                            for mt in range(CT):
                                nc.tensor.matmul(
                                    f2ps[mt][:, :cw],
                                    w2piece[:, kt % HH, mt * 128:(mt + 1) * 128],
                                    h3[:, :cw],
                                    start=(kt == 0), stop=(kt == HT - 1))
                        for mt in range(CT):
                            outc = tmp.tile([128, 290], F16, tag="outc", bufs=3)
                            nc.vector.scalar_tensor_tensor(
                                out=outc[:, :cw], in0=f2ps[mt][:, :cw],
                                scalar=bf2s[:, mt:mt + 1], in1=x2t[:, mt, cs:cs + cw],
                                op0=mybir.AluOpType.add, op1=mybir.AluOpType.add)
                            wout = min(cs + cw, N) - cs
                            nc.sync.dma_start(
                                out=outT[b].rearrange("(kt p) n -> p kt n", p=128)[:, mt, cs:cs + wout],
                                in_=outc[:, :wout])


_NC_CACHE = {}


def _get_nc(b_per_core=B_PER_CORE, num_devices=N_CORES):
    key = (b_per_core, num_devices)
    if key not in _NC_CACHE:
        _NC_CACHE[key] = build_nc(b_per_core, num_devices)
    return _NC_CACHE[key]


def make_in_maps(x, w_qkv, w_proj, b_proj, ln1_g, ln1_b, ln2_g, ln2_b,
                 w_fc1, b_fc1, w_fc2, b_fc2, b_per_core=B_PER_CORE,
                 num_devices=N_CORES):
    f16 = np.float16
    xT = np.ascontiguousarray(
        np.asarray(x, dtype=np.float32).transpose(0, 2, 1)).astype(f16)
    wflat = np.concatenate([
        np.ascontiguousarray(np.asarray(w_qkv, np.float32).T).astype(f16).reshape(-1),
        np.ascontiguousarray(np.asarray(w_proj, np.float32).T).astype(f16).reshape(-1),
        np.ascontiguousarray(np.asarray(w_fc1, np.float32).T).astype(f16).reshape(-1),
        np.ascontiguousarray(np.asarray(w_fc2, np.float32).T).astype(f16).reshape(-1),
    ])
    assert wflat.shape[0] == WTOT
    wshards = wflat.reshape(num_devices, -1)
    cvec = np.concatenate(
        [np.asarray(v, np.float32).reshape(-1, 128).T
         for v in (ln1_g, ln1_b, ln2_g, ln2_b, b_proj, b_fc2, b_fc1)],
        axis=1).astype(f16)
    negoff = np.concatenate([np.eye(3) - 1.0, np.zeros((3, 1))], 1).astype(f16)
    tail = np.concatenate([cvec.reshape(-1), negoff.reshape(-1),
                           np.ones(256, f16)])
    return [
        {"xa": np.ascontiguousarray(
            xT[i * b_per_core:(i + 1) * b_per_core].reshape(-1)),
         "blob2": np.concatenate([wshards[i], tail])}
        for i in range(num_devices)
    ]


def kernel(x, w_qkv, w_proj, b_proj, ln1_g, ln1_b, ln2_g, ln2_b,
           w_fc1, b_fc1, w_fc2, b_fc2):
    nc = _get_nc()
    in_maps = make_in_maps(x, w_qkv, w_proj, b_proj, ln1_g, ln1_b, ln2_g, ln2_b,
                           w_fc1, b_fc1, w_fc2, b_fc2)
    res = run_bass_kernel_spmd(nc, in_maps, core_ids=list(range(N_CORES)))
    outT = np.concatenate([r["outT"] for r in res.results], axis=0)  # [B, C, N] f16
    return np.ascontiguousarray(outT.transpose(0, 2, 1)).astype(np.float32)


# revision 20
# speedup vs baseline: 2.3709x; 1.0237x over previous
"""Trainium2 Bass kernel for a ViT-style transformer block (B=32,N=577,C=768,H=12,HID=3072).

Strategy:
- Data-parallel over batch: 32 batches -> 8 cores x 4 batches.
- The execution path here (axon-tunneled PJRT) re-streams every input
  argument on every call at ~11.6 GB/s, so wall-clock per iteration is
  dominated by argument bytes, not device compute (~0.7ms on-device per the
  cost model). Two levers drive the speedup vs the f32 baseline:
    * Everything shippable is float16 (x, weights, output): rel-rounding
      ~1e-3 against a 2e-2 budget.
    * Weights are shipped sharded 1/8-per-core and AllGathered on-device
      (DRAM->DRAM collective over NeuronLink), removing the 8x data-parallel
      weight replication from the wire.
- Channel-major layout on-chip end-to-end: host pre-transposes x per batch to
  [C, N] and the weights to [K, M]; the output comes back channel-major and is
  transposed on host. This removes every on-chip transpose:
    * LayerNorm over C becomes a ones-vector matmul partition-reduction, with
      the per-token mean/rstd broadcast back across partitions via a K=1 matmul.
    * Attention computes S^T = K^T_slice . Q (keys on partitions), softmax'd
      column-wise: exp on ACT (no max subtraction needed -- |S*scale| < ~3),
      denominators via an appended ones-column on V, normalization folded into
      the PSUM->SBUF eviction against a K=1-broadcast reciprocal row.
    * The post-softmax task mask (3x3 identity block) is applied as a tiny
      rank-3 correction matmul inside the same PSUM accumulation group.
- All matmuls run fp16 x fp16 -> f32 PSUM (full PE rate, 1 cycle/row).
"""

import numpy as np

import concourse.bacc as bacc
import concourse.tile as tile
from concourse import mybir
from concourse.bass_utils import run_bass_kernel_spmd

F32 = mybir.dt.float32
F16 = mybir.dt.float16

B = 32
N = 577
C = 768
H = 12
D = 64
HID = 3072
EPS = 1e-5
SCALE = D ** -0.5

N_CORES = 8
B_PER_CORE = B // N_CORES
CT = C // 128          # 6 channel k-tiles
HT = HID // 128        # 24 hidden k-tiles
NP = 578               # token free-dim padded to even
CHUNKS = [(0, 290), (290, 288)]                    # even free-dim split of NP
MTS = [(0, 128), (128, 128), (256, 128), (384, 128), (512, 65)]  # key m-tiles (real 577)

# flat fp16 weight buffer layout (AllGathered on-device from 1/8 shards),
# split into an early (qkv+proj) and a late (fc1+fc2) gather so the first
# unblocks the attention weight loads while the second rides under compute
WQKV = C * 3 * C
WPROJ = C * C
WFC1 = C * HID
WFC2 = HID * C
WA = WQKV + WPROJ
WB = WFC1 + WFC2
WTOT = WA + WB
WSA = WA // N_CORES
WSB = WB // N_CORES
WSH = WTOT // N_CORES


def _layernorm_cm(nc, ps, tmp, small, src, dst, g_sb, b_sb, mm_bufs=3):
    """Channel-major layernorm: src/dst fp16 [128, CT, N]."""
    musb32 = small.tile([1, NP], F32, tag="musb32", bufs=1)
    musb = small.tile([1, NP], F16, tag="musb", bufs=1)
    varsb = small.tile([1, NP], F32, tag="varsb", bufs=1)
    rstd = small.tile([1, NP], F16, tag="rstdsb", bufs=1)
    ones_k = nc._ones_k

    for cs, cw in CHUNKS:
        sum_ps = ps.tile([1, 290], F32, tag="mm", bufs=mm_bufs)
        sq_ps = ps.tile([1, 290], F32, tag="mm", bufs=mm_bufs)
        for kt in range(CT):
            nc.tensor.matmul(sum_ps[:, :cw], ones_k, src[:, kt, cs:cs + cw],
                             start=(kt == 0), stop=(kt == CT - 1))
            xsq = tmp.tile([128, 290], F16, tag="xsq", bufs=2)
            nc.vector.tensor_mul(xsq[:, :cw], src[:, kt, cs:cs + cw], src[:, kt, cs:cs + cw])
            nc.tensor.matmul(sq_ps[:, :cw], ones_k, xsq[:, :cw],
                             start=(kt == 0), stop=(kt == CT - 1))
        nc.vector.tensor_scalar_mul(musb32[:, cs:cs + cw], sum_ps[:, :cw], 1.0 / C)
        nc.vector.tensor_copy(out=musb[:, cs:cs + cw], in_=musb32[:, cs:cs + cw])
        nc.vector.tensor_mul(varsb[:, cs:cs + cw], musb32[:, cs:cs + cw], musb32[:, cs:cs + cw])
        # var = sq/C - mu^2   (in-place: varsb holds mu^2)
        nc.vector.scalar_tensor_tensor(
            out=varsb[:, cs:cs + cw], in0=sq_ps[:, :cw], scalar=1.0 / C,
            in1=varsb[:, cs:cs + cw], op0=mybir.AluOpType.mult, op1=mybir.AluOpType.subtract)
    for cs, cw in CHUNKS:
        # rstd = 1/sqrt(var + eps), per chunk so chunk 0 unblocks early
        nc.scalar.activation(out=varsb[:, cs:cs + cw], in_=varsb[:, cs:cs + cw],
                             func=mybir.ActivationFunctionType.Sqrt,
                             bias=nc._epst[0:1, :], scale=1.0)
        nc.vector.reciprocal(out=rstd[:, cs:cs + cw], in_=varsb[:, cs:cs + cw])
        mu_ps = ps.tile([128, 290], F32, tag="mm", bufs=mm_bufs)
        rs_ps = ps.tile([128, 290], F32, tag="mm", bufs=mm_bufs)
        nc.tensor.matmul(mu_ps[:, :cw], nc._ones_b, musb[:, cs:cs + cw],
                         start=True, stop=True)
        nc.tensor.matmul(rs_ps[:, :cw], nc._ones_b, rstd[:, cs:cs + cw],
                         start=True, stop=True)
        for kt in range(CT):
            a = tmp.tile([128, 290], F32, tag="lna", bufs=2)
            nc.vector.tensor_sub(a[:, :cw], src[:, kt, cs:cs + cw], mu_ps[:, :cw])
            # (a * g) * rstd
            nc.vector.scalar_tensor_tensor(
                out=dst[:, kt, cs:cs + cw], in0=a[:, :cw], scalar=g_sb[:, kt:kt + 1],
                in1=rs_ps[:, :cw], op0=mybir.AluOpType.mult, op1=mybir.AluOpType.mult)
            nc.vector.tensor_scalar_add(dst[:, kt, cs:cs + cw], dst[:, kt, cs:cs + cw],
                                        b_sb[:, kt:kt + 1])


CV = 128 * (6 * CT + HT)
NEG = 12
B2N = WSH + CV + NEG + 256  # weights-shard + constants blob


def build_nc(b_per_core=B_PER_CORE, num_devices=N_CORES):
    nc = bacc.Bacc("TRN2", target_bir_lowering=False, debug=False,
                   num_devices=num_devices)

    # inputs ride in TWO flat fp16 args: the axon-tunneled execute path costs
    # ~1.1ms of handling per argument per call, and any single argument over
    # ~4MiB falls onto a much slower streaming path -- so pack everything into
    # as few sub-4MiB args as possible
    xn = b_per_core * C * N
    xa = nc.dram_tensor("xa", [xn], F16, kind="ExternalInput").ap()
    blob2 = nc.dram_tensor("blob2", [B2N], F16, kind="ExternalInput").ap()
    CN = C * N

    def xslice(b):  # [128, CT, N] channel-major view of batch b
        return xa[b * CN:(b + 1) * CN].rearrange("(kt p n) -> p kt n", p=128, n=N)

    wshardA = blob2[0:WSA]
    wshardB = blob2[WSA:WSH]
    cvec_d = blob2[WSH:WSH + CV].rearrange("(p k) -> p k", p=128)
    negoff_d = blob2[WSH + CV:WSH + CV + NEG].rearrange("(p f) -> p f", p=3)
    onesc_d = blob2[WSH + CV + NEG:WSH + CV + NEG + 128].rearrange("(p f) -> p f", p=128)
    onesr_d = blob2[WSH + CV + NEG + 128:WSH + CV + NEG + 256].rearrange("(p f) -> p f", p=1)
    outT = nc.dram_tensor("outT", [b_per_core, C, N], F16, kind="ExternalOutput").ap()
    x2T = nc.dram_tensor("x2T_scratch", [b_per_core, C, NP], F16).ap()
    wfullA = nc.dram_tensor("wfullA_gather", [WA], F16, addr_space="Shared").ap()
    wfullB = nc.dram_tensor("wfullB_gather", [WB], F16, addr_space="Shared").ap()

    with tile.TileContext(nc) as tc, \
         nc.allow_low_precision(reason="fp16 operands are rounded intentionally"):
        with tc.tile_pool(name="dramp", bufs=1, space="DRAM") as dramp:
            # weights arrive as 1/8 flat shards; AllGathers assemble the full
            # fp16 weight buffers in local DRAM (collectives can't touch I/O
            # tensors directly, hence the bounce copies)
            wbounceA = dramp.tile([WSA], F16)
            nc.gpsimd.dma_start(out=wbounceA[:], in_=wshardA)
            nc.gpsimd.collective_compute(
                "AllGather", mybir.AluOpType.bypass,
                replica_groups=[list(range(num_devices))],
                ins=[wbounceA.opt()], outs=[wfullA])
            wbounceB = dramp.tile([WSB], F16)
            nc.gpsimd.dma_start(out=wbounceB[:], in_=wshardB)
            nc.gpsimd.collective_compute(
                "AllGather", mybir.AluOpType.bypass,
                replica_groups=[list(range(num_devices))],
                ins=[wbounceB.opt()], outs=[wfullB])
            wqkv_v = wfullA[0:WQKV].rearrange("(kt p f) -> p kt f", p=128, f=3 * C)
            wproj_v = wfullA[WQKV:WA].rearrange("(kt p f) -> p kt f", p=128, f=C)
            wfc1_v = wfullB[0:WFC1].rearrange("(kt p f) -> p kt f", p=128, f=HID)
            wfc2_v = wfullB[WFC1:WB].rearrange("(kt p f) -> p kt f", p=128, f=C)
            self_build(nc, tc, b_per_core, xslice, outT, x2T,
                       wqkv_v, wproj_v, wfc1_v, wfc2_v,
                       negoff_d, cvec_d, onesc_d, onesr_d)

    nc.compile()
    return nc


def self_build(nc, tc, b_per_core, xslice, outT, x2T,
               wqkv_v, wproj_v, wfc1_v, wfc2_v,
               negoff_d, cvec_d, onesc_d, onesr_d):
    with tc.tile_pool(name="const", bufs=1) as cst:
        ones_k = cst.tile([128, 1], F16)
        nc.sync.dma_start(out=ones_k, in_=onesc_d)
        ones_b = cst.tile([1, 128], F16)
        nc.sync.dma_start(out=ones_b, in_=onesr_d)
        ones60 = cst.tile([128, 5, H, 1], F32)
        nc.vector.memset(ones60, 1.0)
        negoff = cst.tile([3, 4], F16)   # [eye(3) - 1 | 0], loaded from host
        nc.sync.dma_start(out=negoff, in_=negoff_d)
        zeros_p = cst.tile([128, 1], F32)
        nc.vector.memset(zeros_p, 0.0)
        epst = cst.tile([1, 1], F32)
        nc.vector.memset(epst, EPS)
        nc._ones_k = ones_k
        nc._ones_b = ones_b
        nc._zeros_p = zeros_p
        nc._epst = epst

        cvech = cst.tile([128, 6 * CT + HT], F16)
        nc.sync.dma_start(out=cvech, in_=cvec_d)
        cvec = cst.tile([128, 6 * CT + HT], F32)
        nc.vector.tensor_copy(out=cvec, in_=cvech)
        g1s = cvec[:, 0 * CT:1 * CT]
        b1s = cvec[:, 1 * CT:2 * CT]
        g2s = cvec[:, 2 * CT:3 * CT]
        b2s = cvec[:, 3 * CT:4 * CT]
        bpjs = cvec[:, 4 * CT:5 * CT]
        bf2s = cvec[:, 5 * CT:6 * CT]
        bf1s = cvec[:, 6 * CT:6 * CT + HT]

        # ---------------- Phase 1: attention block ----------------
        fcw = {}
        with tc.tile_pool(name="w1", bufs=1) as w1p:
            with tc.tile_pool(name="ps1", bufs=1, space="PSUM") as ps, \
             tc.tile_pool(name="act1", bufs=1) as act, \
             tc.tile_pool(name="tmp1", bufs=1) as tmp, \
             tc.tile_pool(name="small1", bufs=1) as small:
                # x(b0) first so LN1 starts while weights gather/stream in
                xt0 = act.tile([128, CT, NP], F16, tag="xt", bufs=2, name="xt0")
                nc.vector.memset(xt0[:, :, N:NP], 0.0)
                nc.sync.dma_start(out=xt0[:, :, 0:N], in_=xslice(0))
                # all large weights share one 5-slot rotation; the fc halves
                # reuse the qkv slots once those go dead at the last batch
                wq_sb = w1p.tile([128, CT, C], F16, tag="w", bufs=5, name="wq")
                nc.sync.dma_start(out=wq_sb, in_=wqkv_v[:, :, 0:C])
                wk_sb = w1p.tile([128, CT, C], F16, tag="w", bufs=5, name="wk")
                nc.sync.dma_start(out=wk_sb, in_=wqkv_v[:, :, C:2 * C])
                wv_sb = w1p.tile([128, CT, C], F16, tag="w", bufs=5, name="wv")
                nc.sync.dma_start(out=wv_sb, in_=wqkv_v[:, :, 2 * C:3 * C])
                wproj_sb = w1p.tile([128, CT, C], F16, tag="w", bufs=5, name="wproj")
                nc.sync.dma_start(out=wproj_sb, in_=wproj_v)

                def emit_xload(b):
                    xt_ = act.tile([128, CT, NP], F16, tag="xt", bufs=2,
                                   name=f"xt_b{b}")
                    nc.vector.memset(xt_[:, :, N:NP], 0.0)
                    nc.sync.dma_start(out=xt_[:, :, 0:N], in_=xslice(b))
                    return xt_

                def emit_ln1(b, xt_):
                    ht_ = act.tile([128, CT, NP], F16, tag="ht", bufs=1,
                                   name=f"ht_b{b}")
                    _layernorm_cm(nc, ps, tmp, small, xt_, ht_, g1s, b1s)
                    return ht_

                from collections import deque

                def emit_qk_mt(ht_, qk_, mt):
                    for cs, cw in CHUNKS:
                        mm = ps.tile([128, 290], F32, tag="mm", bufs=3)
                        wqk = wq_sb if mt < CT else wk_sb
                        fo = (mt % CT) * 128
                        for kt in range(CT):
                            nc.tensor.matmul(
                                mm[:, :cw],
                                wqk[:, kt, fo:fo + 128],
                                ht_[:, kt, cs:cs + cw],
                                start=(kt == 0), stop=(kt == CT - 1))
                        if mt < CT:  # q: fold in softmax scale
                            if mt % 2 == 0:
                                nc.scalar.mul(out=qk_[:, mt, cs:cs + cw],
                                              in_=mm[:, :cw], mul=SCALE)
                            else:
                                nc.vector.tensor_scalar_mul(
                                    qk_[:, mt, cs:cs + cw], mm[:, :cw], SCALE)
                        else:
                            if mt % 2 == 0:
                                nc.scalar.copy(out=qk_[:, mt, cs:cs + cw],
                                               in_=mm[:, :cw])
                            else:
                                nc.vector.tensor_copy(
                                    out=qk_[:, mt, cs:cs + cw], in_=mm[:, :cw])

                def emit_v_mt(ht_, vaug_, imt):
                    ms, mw = MTS[imt]
                    for j in range(2):
                        vm = ps.tile([128, 384], F32, tag="mm", bufs=3)
                        for kt in range(CT):
                            nc.tensor.matmul(
                                vm[:mw, :],
                                ht_[:, kt, ms:ms + mw],
                                wv_sb[:, kt, 384 * j:384 * (j + 1)],
                                start=(kt == 0), stop=(kt == CT - 1))
                        if (imt + j) % 2 == 0:
                            nc.scalar.copy(
                                out=vaug_[:mw, imt, 6 * j:6 * (j + 1), 0:D],
                                in_=vm[:mw, :].rearrange("p (h d) -> p h d", d=D))
                        else:
                            nc.vector.tensor_copy(
                                out=vaug_[:mw, imt, 6 * j:6 * (j + 1), 0:D],
                                in_=vm[:mw, :].rearrange("p (h d) -> p h d", d=D))

                def alloc_qk(b_):
                    return act.tile([128, 2 * CT, NP], F16,
                                    tag="qk", bufs=2, name=f"qk_b{b_}")

                def alloc_vaug(b_):
                    v_ = act.tile([128, 5, H, D + 1], F16,
                                  tag="vaug", bufs=2, name=f"vaug_b{b_}")
                    nc.vector.tensor_copy(out=v_[:, :, :, D:D + 1], in_=ones60)
                    return v_

                # state carried across batches: (xt, ht, qk, vaug)
                xts = {0: xt0}
                pre = {0: emit_ln1(0, xt0)}
                qks, vaugs = {}, {}
                qks[0] = alloc_qk(0)
                for mt in range(2 * CT):
                    emit_qk_mt(pre[0], qks[0], mt)
                vaugs[0] = alloc_vaug(0)
                for imt in range(len(MTS)):
                    emit_v_mt(pre[0], vaugs[0], imt)

                for b in range(b_per_core):
                    ht = pre.pop(b)
                    xt = xts[b]
                    qk = qks.pop(b)
                    vaug = vaugs.pop(b)
                    if b + 1 < b_per_core:
                        xts[b + 1] = emit_xload(b + 1)

                    # hooks: next batch's LN/qk/v emission interleaved
                    # between this batch's attention heads
                    hooks = {}
                    if b + 1 < b_per_core:
                        def mk(fn, *args):
                            return lambda: fn(*args)
                        def hook_ln():
                            pre[b + 1] = emit_ln1(b + 1, xts[b + 1])
                        def hook_qk_alloc():
                            qks[b + 1] = alloc_qk(b + 1)
                        def hook_vaug_alloc():
                            vaugs[b + 1] = alloc_vaug(b + 1)
                        hooks[0] = [hook_ln, hook_qk_alloc]
                        for h_ in range(1, 7):
                            hooks[h_] = [mk(lambda mt_: emit_qk_mt(pre[b + 1], qks[b + 1], mt_), m)
                                         for m in (2 * (h_ - 1), 2 * (h_ - 1) + 1)]
                        hooks[7] = [hook_vaug_alloc,
                                    mk(lambda i_: emit_v_mt(pre[b + 1], vaugs[b + 1], i_), 0)]
                        hooks[8] = [mk(lambda i_: emit_v_mt(pre[b + 1], vaugs[b + 1], i_), i) for i in (1, 2)]
                        hooks[9] = [mk(lambda i_: emit_v_mt(pre[b + 1], vaugs[b + 1], i_), i) for i in (3, 4)]

                    if b == b_per_core - 1:
                        # qkv weights dead (next batch's qk/v already emitted);
                        # stream the fc weights into their slots under this
                        # attention + proj
                        NQ = 2
                        HH = HT // NQ
                        fcw["wfc1"] = [w1p.tile([128, CT, HID // NQ], F16,
                                                tag="w", bufs=5, name=f"wfc1_{i}")
                                       for i in range(NQ)]
                        fcw["wfc2"] = [w1p.tile([128, HH, C], F16,
                                                tag="w", bufs=5, name=f"wfc2_{i}")
                                       for i in range(NQ)]
                        fcw["HH"] = HH
                        for i in range(NQ):
                            lo, hi = i * (HID // NQ), (i + 1) * (HID // NQ)
                            nc.sync.dma_start(out=fcw["wfc1"][i], in_=wfc1_v[:, :, lo:hi])
                            nc.sync.dma_start(out=fcw["wfc2"][i],
                                              in_=wfc2_v[:, i * HH:(i + 1) * HH, :])

                    # attention, head by head; output channel-major into oT
                    oT = act.tile([128, CT, NP], F16, tag="oT", bufs=1)
                    pend = deque()

                    def push(fn, lag=3):
                        pend.append(fn)
                        while len(pend) > lag:
                            pend.popleft()()

                    state = {}

                    def make_o(h, imt, pt):
                        ms, mw = MTS[imt]
                        def f():
                            if "o_ps" not in state[h]:
                                state[h]["o_ps"] = [
                                    ps.tile([D + 1, 290], F32, tag="oacc", bufs=2,
                                            name=f"ops_b{b}h{h}c{ci_}")
                                    for ci_ in range(2)]
                            o_ps = state[h]["o_ps"]
                            for ci, (cs, cw) in enumerate(CHUNKS):
                                last = (imt == len(MTS) - 1)
                                nc.tensor.matmul(
                                    o_ps[ci][:, :cw],
                                    vaug[:mw, imt, h, :],
                                    pt[:mw, cs:cs + cw],
                                    start=(imt == 0), stop=last)
                            if imt == 0:
                                # task-mask correction rides inside the same
                                # accumulation group (order is commutative)
                                nc.tensor.matmul(
                                    o_ps[0][0:D, 0:4], vaug[0:3, 0, h, 0:D],
                                    state[h]["tmp33"], start=False, stop=False)
                        return f

                    def make_fin(h):
                        grp, qb = h // 2, 64 * (h % 2)
                        def f():
                            o_ps = state[h]["o_ps"]
                            for ci, (cs, cw) in enumerate(CHUNKS):
                                rsb = small.tile([1, 290], F16, tag="rsb", bufs=2)
                                nc.vector.reciprocal(out=rsb[:, :cw],
                                                     in_=o_ps[ci][D:D + 1, :cw])
                                rp = ps.tile([64, 290], F32, tag="st", bufs=3)
                                nc.tensor.matmul(rp[:, :cw], ones_b[0:1, 0:D],
                                                 rsb[:, :cw], start=True, stop=True)
                                rps = tmp.tile([64, 290], F32, tag="rps", bufs=3)
                                nc.scalar.copy(out=rps[:, :cw], in_=rp[:, :cw])
                                nc.vector.tensor_mul(oT[qb:qb + D, grp, cs:cs + cw],
                                                     o_ps[ci][0:D, :cw], rps[:, :cw])
                        return f

                    for h in range(H):
                        grp, qb = h // 2, 64 * (h % 2)
                        state[h] = {}
                        for imt, (ms, mw) in enumerate(MTS):
                            pt = tmp.tile([128, NP], F16, tag="pt", bufs=4)
                            for ci, (cs, cw) in enumerate(CHUNKS):
                                st = ps.tile([128, 290], F32, tag="st", bufs=3)
                                nc.tensor.matmul(
                                    st[:mw, :cw],
                                    qk[qb:qb + D, CT + grp, ms:ms + mw],
                                    qk[qb:qb + D, grp, cs:cs + cw],
                                    start=True, stop=True)
                                nc.scalar.activation(
                                    out=pt[:mw, cs:cs + cw], in_=st[:mw, :cw],
                                    func=mybir.ActivationFunctionType.Exp,
                                    bias=zeros_p[:mw, :], scale=1.0)
                            if imt == 0:
                                tmp33 = small.tile([3, 4], F16, tag="t33", bufs=2)
                                nc.vector.tensor_mul(tmp33, pt[0:3, 0:4], negoff)
                                state[h]["tmp33"] = tmp33
                            push(make_o(h, imt, pt))
                        push(make_fin(h))
                        for fn in hooks.get(h, []):
                            fn()
                    while pend:
                        pend.popleft()()

                    # proj + bias + residual -> x2T (DRAM scratch)
                    for mt in range(CT):
                        for cs, cw in CHUNKS:
                            mm = ps.tile([128, 290], F32, tag="mm", bufs=3)
                            for kt in range(CT):
                                nc.tensor.matmul(
                                    mm[:, :cw],
                                    wproj_sb[:, kt, mt * 128:(mt + 1) * 128],
                                    oT[:, kt, cs:cs + cw],
                                    start=(kt == 0), stop=(kt == CT - 1))
                            x2c = tmp.tile([128, 290], F16, tag="x2c", bufs=2)
                            nc.vector.scalar_tensor_tensor(
                                out=x2c[:, :cw], in0=mm[:, :cw],
                                scalar=bpjs[:, mt:mt + 1], in1=xt[:, mt, cs:cs + cw],
                                op0=mybir.AluOpType.add, op1=mybir.AluOpType.add)
                            nc.sync.dma_start(
                                out=x2T[b].rearrange("(kt p) n -> p kt n", p=128)[:, mt, cs:cs + cw],
                                in_=x2c[:, :cw])

            # ---------------- Phase 2: MLP block ----------------
            with tc.tile_pool(name="ps2", bufs=1, space="PSUM") as ps, \
             tc.tile_pool(name="act2", bufs=1) as act, \
             tc.tile_pool(name="tmp2", bufs=1) as tmp, \
             tc.tile_pool(name="small2", bufs=1) as small:
                wfc1_sb, wfc2_sb, HH = fcw["wfc1"], fcw["wfc2"], fcw["HH"]
                x2t0 = act.tile([128, CT, NP], F16, tag="x2t", bufs=2, name="x2t0")
                nc.sync.dma_start(out=x2t0, in_=x2T[0].rearrange("(kt p) n -> p kt n", p=128))
                h2t0 = act.tile([128, CT, NP], F16, tag="h2t", bufs=2, name="h2t0")
                _layernorm_cm(nc, ps, tmp, small, x2t0, h2t0, g2s, b2s, mm_bufs=2)

                def emit_x2load(b):
                    x2t_ = act.tile([128, CT, NP], F16, tag="x2t", bufs=2,
                                    name=f"x2t_b{b}")
                    nc.sync.dma_start(out=x2t_, in_=x2T[b].rearrange("(kt p) n -> p kt n", p=128))
                    return x2t_

                def emit_ln2(b, x2t_):
                    h2t_ = act.tile([128, CT, NP], F16, tag="h2t",
                                    bufs=2, name=f"h2t_b{b}")
                    _layernorm_cm(nc, ps, tmp, small, x2t_, h2t_, g2s, b2s, mm_bufs=2)
                    return h2t_

                x2ts = {0: x2t0}
                pre2 = {0: (x2t0, h2t0)}
                for b in range(b_per_core):
                    x2t, h2t = pre2.pop(b)
                    if b + 1 < b_per_core:
                        x2ts[b + 1] = emit_x2load(b + 1)

                    for ci_chunk, (cs, cw) in enumerate(CHUNKS):
                        # LN2 of the next batch rides under this batch's last chunk
                        if ci_chunk == 1 and b + 1 < b_per_core:
                            pre2[b + 1] = (x2ts[b + 1], emit_ln2(b + 1, x2ts[b + 1]))
                        f2ps = [ps.tile([128, 290], F32, tag="fc2", bufs=6,
                                        name=f"f2ps_b{b}c{cs}m{mt_}")
                                for mt_ in range(CT)]
                        for kt in range(HT):
                            f1 = ps.tile([128, 290], F32, tag="mm", bufs=2)
                            w1piece = wfc1_sb[kt // HH]
                            ko = (kt % HH) * 128
                            for ct in range(CT):
                                nc.tensor.matmul(
                                    f1[:, :cw],
                                    w1piece[:, ct, ko:ko + 128],
                                    h2t[:, ct, cs:cs + cw],
                                    start=(ct == 0), stop=(ct == CT - 1))
                            h3 = tmp.tile([128, 290], F16, tag="h3", bufs=3)
                            nc.scalar.activation(
                                out=h3[:, :cw], in_=f1[:, :cw],
                                func=mybir.ActivationFunctionType.Gelu,
                                bias=bf1s[:, kt:kt + 1], scale=1.0)
                            w2piece = wfc2_sb[kt // HH]
                            for mt in range(CT):
                                nc.tensor.matmul(
                                    f2ps[mt][:, :cw],
                                    w2piece[:, kt % HH, mt * 128:(mt + 1) * 128],
                                    h3[:, :cw],
                                    start=(kt == 0), stop=(kt == HT - 1))
                        for mt in range(CT):
                            outc = tmp.tile([128, 290], F16, tag="outc", bufs=3)
                            nc.vector.scalar_tensor_tensor(
                                out=outc[:, :cw], in0=f2ps[mt][:, :cw],
                                scalar=bf2s[:, mt:mt + 1], in1=x2t[:, mt, cs:cs + cw],
                                op0=mybir.AluOpType.add, op1=mybir.AluOpType.add)
                            wout = min(cs + cw, N) - cs
                            nc.sync.dma_start(
                                out=outT[b].rearrange("(kt p) n -> p kt n", p=128)[:, mt, cs:cs + wout],
                                in_=outc[:, :wout])


_NC_CACHE = {}


def _get_nc(b_per_core=B_PER_CORE, num_devices=N_CORES):
    key = (b_per_core, num_devices)
    if key not in _NC_CACHE:
        _NC_CACHE[key] = build_nc(b_per_core, num_devices)
    return _NC_CACHE[key]


def make_in_maps(x, w_qkv, w_proj, b_proj, ln1_g, ln1_b, ln2_g, ln2_b,
                 w_fc1, b_fc1, w_fc2, b_fc2, b_per_core=B_PER_CORE,
                 num_devices=N_CORES):
    f16 = np.float16
    xT = np.ascontiguousarray(
        np.asarray(x, dtype=np.float32).transpose(0, 2, 1)).astype(f16)
    wflat = np.concatenate([
        np.ascontiguousarray(np.asarray(w_qkv, np.float32).T).astype(f16).reshape(-1),
        np.ascontiguousarray(np.asarray(w_proj, np.float32).T).astype(f16).reshape(-1),
        np.ascontiguousarray(np.asarray(w_fc1, np.float32).T).astype(f16).reshape(-1),
        np.ascontiguousarray(np.asarray(w_fc2, np.float32).T).astype(f16).reshape(-1),
    ])
    assert wflat.shape[0] == WTOT
    wshardsA = wflat[:WA].reshape(num_devices, -1)
    wshardsB = wflat[WA:].reshape(num_devices, -1)
    cvec = np.concatenate(
        [np.asarray(v, np.float32).reshape(-1, 128).T
         for v in (ln1_g, ln1_b, ln2_g, ln2_b, b_proj, b_fc2, b_fc1)],
        axis=1).astype(f16)
    negoff = np.concatenate([np.eye(3) - 1.0, np.zeros((3, 1))], 1).astype(f16)
    tail = np.concatenate([cvec.reshape(-1), negoff.reshape(-1),
                           np.ones(256, f16)])
    return [
        {"xa": np.ascontiguousarray(
            xT[i * b_per_core:(i + 1) * b_per_core].reshape(-1)),
         "blob2": np.concatenate([wshardsA[i], wshardsB[i], tail])}
        for i in range(num_devices)
    ]


def kernel(x, w_qkv, w_proj, b_proj, ln1_g, ln1_b, ln2_g, ln2_b,
           w_fc1, b_fc1, w_fc2, b_fc2):
    nc = _get_nc()
    in_maps = make_in_maps(x, w_qkv, w_proj, b_proj, ln1_g, ln1_b, ln2_g, ln2_b,
                           w_fc1, b_fc1, w_fc2, b_fc2)
    res = run_bass_kernel_spmd(nc, in_maps, core_ids=list(range(N_CORES)))
    outT = np.concatenate([r["outT"] for r in res.results], axis=0)  # [B, C, N] f16
    return np.ascontiguousarray(outT.transpose(0, 2, 1)).astype(np.float32)
